# revision 38
# baseline (speedup 1.0000x reference)
"""Causal self-attention Trainium2 Bass kernel (V5).

Full-input contract: kernel(**inputs) takes the unsharded inputs
(x [8,1024,768], W_attn [768,2304], b_attn [2304], W_proj [768,768],
b_proj [768]) and returns the full output [8,1024,768].

Sharding: data parallel - batch element b runs on NeuronCore b (B=8 =
n_cores), no collectives needed.

V5 changes vs V4 (trace-driven; V4 331us, PE 60% cov, HAM throttled to
K=4/8 for the entire 188us attention phase, 40us DVE reciprocal, 88us
ACT exp):
  - host-side prep: x is transposed and cast to bf16 on the host
    (xT input [768,1024]); W_attn split into Wq/Wk/Wv and cast bf16;
    W_proj bf16. Kills the 48 PE transposes + DVE copies of phase 1 and
    halves weight DMA bytes.
  - all GEMMs run on bf16 operands (fp32 PSUM accumulation).
  - attention pipeline unit is a (k-tile, col-half) CHUNK with fp32
    scores in a [128, 2x512] PSUM tile (2 banks, both heads of the
    pair). Chunks are double-buffered (4 banks) next to the 4 avp
    banks, so the score MM for chunk n+2 no longer waits on exp(n):
    the PE never idles long enough for HAM to re-throttle.
  - one exp per chunk covers both heads ([128, 2, n] AP) - halves ACT
    instruction count; one affine_select masks both heads' diagonal.
  - softmax divide: l-rows gathered by SBUF->SBUF DMA into a [4,512]
    tile per head pair, ONE reciprocal_approx_fast (the V4 kernel spent
    40us in 12 full-precision Newton reciprocals), DMA broadcast,
    DVE multiply fused into yT (bf16).
"""

import os
import sys

import numpy as np

for _p in ("/opt/trn_rl_repo", "/root/.axon_site/_ro/trn_rl_repo"):
    if os.path.isdir(_p) and _p not in sys.path:
        sys.path.insert(0, _p)
        break

import concourse.bass as bass
import concourse.mybir as mybir
import concourse.tile as tile
from concourse.bass_utils import run_bass_kernel_spmd

T, C, H = 1024, 768, 12
C3 = 3 * C
NCORES = 8
NT = T // 128    # 8 t-tiles
NC_ = C // 128   # 6 c-tiles
NHP = H // 2     # 6 head pairs
f32 = mybir.dt.float32
bf16 = mybir.dt.bfloat16

EXP = mybir.ActivationFunctionType.Exp


def build_module():
    nc = bass.Bass()
    xT_d = nc.dram_tensor("xT", [C, T], bf16, kind="ExternalInput")
    wqk_d = nc.dram_tensor("Wqk", [C, 2 * C], bf16, kind="ExternalInput")
    wv_d = nc.dram_tensor("Wv", [C, C], bf16, kind="ExternalInput")
    wp_d = nc.dram_tensor("Wp", [C, C], bf16, kind="ExternalInput")
    ba_d = nc.dram_tensor("b_attn", [1, C3], f32, kind="ExternalInput")
    bp_d = nc.dram_tensor("b_proj", [1, C], f32, kind="ExternalInput")
    out_d = nc.dram_tensor("out", [T, C], f32, kind="ExternalOutput")

    with tile.TileContext(nc) as tc:
        with tc.tile_pool(name="persist", bufs=1) as P0:
            qkT = [P0.tile([128, T], bf16, name=f"qkT{m}") for m in range(2 * NC_)]
            # per head: 64 v-dim columns + 64 ones-columns. The AV matmul
            # then emits y rows at partitions 0:64 AND the softmax
            # denominator l replicated across partitions 64:128 - a free
            # partition-broadcast on the PE (MM cost depends only on the
            # moving-operand columns).
            vA = [P0.tile([128, 128 * H], bf16, name=f"vA{t}") for t in range(NT)]
            yT = [P0.tile([128, T], bf16, name=f"yT{c}") for c in range(NC_)]
            ba_sb = P0.tile([1, C], f32, name="ba_sb")
            bp_sb = P0.tile([1, C], f32, name="bp_sb")
            baB = P0.tile([128, C], f32, name="baB")   # b_attn v-part bcast
            bpB = P0.tile([128, C], f32, name="bpB")   # b_proj bcast
            wpt = [P0.tile([128, C], bf16, name=f"wp{c}") for c in range(NC_)]
            bqk = [P0.tile([128, 1], f32, name=f"bqk{m}") for m in range(2 * NC_)]
            # xT / q|k weights persist into the attention phase: the q^T/k^T
            # GEMM for head pair hp+1 is interleaved into hp's attention
            xT = [P0.tile([128, T], bf16, name=f"xT{c}") for c in range(NC_)]
            wQK = [P0.tile([128, 2 * C], bf16, name=f"wQK{c}")
                   for c in range(NC_)]
            warm_src = P0.tile([1, 16], f32, name="warm_src")
            nc.vector.memset(warm_src[:], 1.0)

            # preload the exp table while ACT is idle (else the first
            # attention exp pays the ~2.7us ACT_TABLE_LOAD inline)
            warm = P0.tile([1, 16], f32, name="warm")
            nc.scalar.activation(warm[:], warm_src[:], EXP, scale=0.125)

            # ---- phase A: v GEMM (x arrives pre-transposed bf16) ----
            with tc.tile_pool(name="sbA", bufs=1) as SBA:
                wV = [SBA.tile([128, C], bf16, name=f"wV{c}", tag=f"wV{c}",
                               bufs=1) for c in range(NC_)]
                # interleave x/weight loads across both HWDGE queues so the
                # first v-GEMM accumulation chain can start ~2 tiles in
                nc.sync.dma_start(out=ba_sb[:], in_=ba_d[0:1, 2 * C:3 * C])
                for c in range(NC_):
                    q = nc.sync if c % 2 == 0 else nc.scalar
                    q2 = nc.scalar if c % 2 == 0 else nc.sync
                    q.dma_start(out=xT[c][:],
                                in_=xT_d[128 * c:128 * (c + 1), :])
                    q2.dma_start(out=wV[c][:],
                                 in_=wv_d[128 * c:128 * (c + 1), :])
                # one-time bias broadcast (free-dim stride-0 DMA replicate)
                # on the gpsimd SWDGE queue (descriptor gen on Q7, off both
                # HWDGE queues)
                nc.gpsimd.dma_start(
                    out=baB[:],
                    in_=ba_sb[0:1, :].unsqueeze(1).to_broadcast([1, 128, C]))
                baB_r = baB.rearrange("p (h e) -> p h e", h=H)
                # bqk partition-scatter DMAs (4B-granular, slow to issue) on
                # the gpsimd SWDGE queue, off the weight-load path
                for m in range(2 * NC_):
                    nc.gpsimd.dma_start(
                        out=bqk[m][:],
                        in_=ba_d[0:1, 128 * m:128 * (m + 1)]
                            .rearrange("a p -> p a"))
                # q|k weight loads stream behind the v weights; two DMAs per
                # tile - a single [128,1536] load (3KB rows) hits a slow
                # descriptor path (>10us issue), 1.5KB rows issue in ~0.6us
                for c in range(NC_):
                    q = nc.sync if c % 2 == 0 else nc.scalar
                    q2 = nc.scalar if c % 2 == 0 else nc.sync
                    q.dma_start(out=wQK[c][:, 0:C],
                                in_=wqk_d[128 * c:128 * (c + 1), 0:C])
                    q2.dma_start(out=wQK[c][:, C:2 * C],
                                 in_=wqk_d[128 * c:128 * (c + 1), C:2 * C])

                with tc.tile_pool(name="psA", bufs=1, space="PSUM") as PSA:
                    # v: stationary xT columns, moving W_v rows. bufs=4 (the
                    # whole PSUM): every t-tile needs ALL SIX xT/wV tiles, so
                    # until the last weight DMA lands only partial-product
                    # chains can run - 4 accumulators let 4 t-tiles' partials
                    # proceed DMA-paced instead of serializing at the end.
                    for t in range(NT):
                        accv = PSA.tile([128, C], f32, tag="v", bufs=4,
                                        name="accv")
                        for c in range(NC_):
                            xcol = xT[c][:, 128 * t:128 * (t + 1)]
                            nc.tensor.matmul(accv[:, 0:512], xcol,
                                             wV[c][:, 0:512],
                                             start=(c == 0), stop=(c == NC_ - 1))
                            nc.tensor.matmul(accv[:, 512:C], xcol,
                                             wV[c][:, 512:C],
                                             start=(c == 0), stop=(c == NC_ - 1))
                        # per-head layout [ones(64) | v(64)]: the ones FIRST
                        # so the AV matmul puts the replicated l at
                        # partitions 0:64 - the custom-DVE reciprocal ignores
                        # a shifted input partition base, standard TT doesn't
                        av = vA[t].rearrange("p (h e) -> p h e", h=H)
                        nc.vector.memset(av[:, :, 0:64], 1.0)
                        # eviction with fused bias add
                        nc.vector.tensor_tensor(
                            av[:, :, 64:128],
                            accv[:].rearrange("p (h e) -> p h e", h=H),
                            baB_r[:, :, 0:64],
                            mybir.AluOpType.add)

            # ---- phase B: attention with interleaved q^T/k^T GEMMs ----
            with tc.tile_pool(name="ps3", bufs=1, space="PSUM") as PS3, \
                 tc.tile_pool(name="sb3", bufs=1) as SB3:
                from collections import deque
                pending = deque()   # deferred normalization pipeline stages

                def pop_pending(k=2):
                    n = 0
                    while pending and n < k:
                        s = pending.popleft()
                        if s is not None:
                            s()
                        n += 1

                # chunk list: (i, w); w=0 -> query cols [lo,512) (i<4 only),
                # w=1 -> [max(lo,512), 1024). L chunks first so the L-half
                # finishes early - its normalization frees the avs L banks
                # for the interleaved qk jobs with slack to spare.
                chunks = [(i, 0) for i in range(4)] + \
                         [(i, 1) for i in range(NT)]

                def chunk_cols(i, w):
                    lo = 128 * i
                    if w == 0:
                        return lo, 512
                    return max(lo, 512), T

                def emit_qk_job(tp, jidx, tags):
                    # one [128,512] slice of q^T (jidx 0/1) or k^T (2/3) for
                    # target head pair tp; the accumulator borrows an
                    # avs-tagged PSUM bank (free between L-normalization and
                    # the next pair's AV allocation)
                    m = tp if jidx < 2 else NC_ + tp
                    j2 = jidx % 2
                    acc = PS3.tile([128, 512], f32, tag=tags[jidx], bufs=1,
                                   name="qka")
                    for c in range(NC_):
                        nc.tensor.matmul(
                            acc[:], wQK[c][:, 128 * m:128 * (m + 1)],
                            xT[c][:, 512 * j2:512 * (j2 + 1)],
                            start=(c == 0), stop=(c == NC_ - 1))
                    # psum -> sbuf(bf16) with per-partition bias add
                    nc.vector.tensor_scalar_add(
                        qkT[m][:, 512 * j2:512 * (j2 + 1)], acc[:],
                        bqk[m][:])

                sps = {}    # (hp, ch) -> score PSUM tile
                pbs = {}    # (hp, ch) -> exp'd SBUF tile
                avst = {}   # (hp, hs, half) -> [128,512] accumulator

                def emit_score(hp, ch):
                    i, w = ch
                    lo = 128 * i
                    c0, c1 = chunk_cols(i, w)
                    qt = qkT[hp]
                    kt = qkT[NC_ + hp]
                    scp = PS3.tile([128, 1024], f32, tag="sc", bufs=2,
                                   name="scp")
                    for hs in range(2):
                        base = 64 * hs
                        nc.tensor.matmul(
                            scp[:, 512 * hs:512 * hs + (c1 - c0)],
                            kt[base:base + 64, lo:lo + 128],
                            qt[base:base + 64, c0:c1],
                            start=True, stop=True)
                    sps[(hp, ch)] = scp

                def emit_exp(hp, ch):
                    i, w = ch
                    lo = 128 * i
                    c0, c1 = chunk_cols(i, w)
                    n = c1 - c0
                    scp = sps.pop((hp, ch))
                    pb = SB3.tile([128, 1024], bf16, tag="pb", bufs=4,
                                  name="pb")
                    scv = scp.rearrange("p (s n) -> p s n", s=2)
                    pbv = pb.rearrange("p (s n) -> p s n", s=2)
                    nc.scalar.activation(pbv[:, :, 0:n], scv[:, :, 0:n],
                                         EXP, scale=0.125)
                    if c0 == lo:
                        # diagonal [128,128] block (both heads):
                        # keep iff q - key >= 0
                        nc.gpsimd.affine_select(
                            out=pbv[:, :, 0:128], in_=pbv[:, :, 0:128],
                            pattern=[[0, 2], [1, 128]],
                            compare_op=mybir.AluOpType.is_ge, fill=0.0,
                            base=0, channel_multiplier=-1,
                        )
                    pbs[(hp, ch)] = pb

                def emit_av(hp, ch):
                    i, w = ch
                    c0, c1 = chunk_cols(i, w)
                    n = c1 - c0
                    pb = pbs.pop((hp, ch))
                    if i == 0 and w == 0:
                        for hs in range(2):
                            for half in range(2):
                                avst[(hp, hs, half)] = PS3.tile(
                                    [128, 512], f32, tag=f"av{hs}{half}",
                                    bufs=1, name=f"av{hs}{half}")
                    for hs in range(2):
                        h = 2 * hp + hs
                        vt = vA[i][:, 128 * h:128 * h + 128]
                        if w == 0:
                            nc.tensor.matmul(
                                avst[(hp, hs, 0)][:, c0:512], vt,
                                pb[:, 512 * hs:512 * hs + n],
                                start=(i == 0), stop=(i == 3),
                                skip_group_check=True)
                        else:
                            nc.tensor.matmul(
                                avst[(hp, hs, 1)][:, c0 - 512:512], vt,
                                pb[:, 512 * hs:512 * hs + n],
                                start=(i == 0), stop=(i == NT - 1),
                                skip_group_check=True)

                def make_norm(hp, half):
                    # avs rows 0:64 hold l replicated across partitions
                    # (ones-columns in vA), rows 64:128 hold y.
                    # reciprocal + normalize read PSUM directly - no
                    # staging copies, no DMA gathers/broadcasts.
                    loc = {}

                    def s_recip():
                        for hs in range(2):
                            rli = SB3.tile([64, 512], f32,
                                           tag=f"rli{hs}{half}", bufs=2,
                                           name=f"rli{hs}{half}")
                            nc.vector.reciprocal_approx_fast(
                                rli[:],
                                avst[(hp, hs, half)][0:64, 0:512])
                            loc[hs] = rli

                    def s_mult():
                        for hs in range(2):
                            base = 64 * hs
                            nc.vector.tensor_tensor(
                                yT[hp][base:base + 64,
                                       512 * half:512 * (half + 1)],
                                avst[(hp, hs, half)][64:128, 0:512],
                                loc[hs][:],
                                mybir.AluOpType.mult)

                    return [s_recip, s_mult]

                # prelude for head pair 0: the L-half chunks only need the
                # j2=0 (query/key cols < 512) jobs, so those go first and
                # the first score/exp pairs interleave with jobs 1 and 3
                JTAGS = ["av00", "av10", "av00", "av10"]
                nch = len(chunks)
                emit_qk_job(0, 0, ["av00"] * 4)
                emit_qk_job(0, 2, ["av10"] * 4)
                emit_score(0, chunks[0])
                emit_exp(0, chunks[0])
                emit_qk_job(0, 1, ["av01"] * 4)
                emit_score(0, chunks[1])
                emit_exp(0, chunks[1])
                emit_qk_job(0, 3, ["av11"] * 4)

                for hp in range(NHP):
                    if hp == 0:
                        # W_proj / b_proj loads: sync-queue only (a scalar-
                        # queue DMA here would head-block the exp stream)
                        nc.sync.dma_start(out=bp_sb[:], in_=bp_d[:])
                        for c in range(NC_):
                            nc.sync.dma_start(
                                out=wpt[c][:],
                                in_=wp_d[128 * c:128 * (c + 1), :])
                        nc.gpsimd.dma_start(
                            out=bpB[:],
                            in_=bp_sb[0:1, :].unsqueeze(1)
                                .to_broadcast([1, 128, C]))
                    # steady state: scores TWO chunks ahead of the AV
                    # stream so the in-order PE queue always has a score MM
                    # to run while AV(n) waits on exp/affine(n). The first
                    # two score/exp pairs were emitted in the previous
                    # pair's tail (or the prelude).
                    for n_ in range(2, nch):
                        pop_pending()
                        emit_score(hp, chunks[n_])
                        emit_exp(hp, chunks[n_])
                        emit_av(hp, chunks[n_ - 2])
                        if n_ == 5:
                            # L-half normalization inline: the avs L banks
                            # are re-tagged as qk accumulators right after
                            for s in make_norm(hp, 0):
                                s()
                        elif n_ in (6, 8, 10) and hp + 1 < NHP:
                            emit_qk_job(hp + 1, {6: 0, 8: 1, 10: 2}[n_],
                                        JTAGS)
                    pop_pending()
                    emit_av(hp, chunks[nch - 2])
                    emit_av(hp, chunks[nch - 1])
                    if hp + 1 < NHP:
                        # next pair's warmup in this tail: keeps ACT fed
                        # across the pair boundary (its inputs - the j2=0
                        # qk jobs - completed mid-pair)
                        emit_score(hp + 1, chunks[0])
                        emit_exp(hp + 1, chunks[0])
                        emit_score(hp + 1, chunks[1])
                        emit_exp(hp + 1, chunks[1])
                        emit_qk_job(hp + 1, 3, JTAGS)
                    pending.extend(make_norm(hp, 1))

                # drain the tail (last pair's normalization)
                while pending:
                    s = pending.popleft()
                    if s is not None:
                        s()

            # ---- phase C: out = y^T.T @ W_proj + b_proj ----
            with tc.tile_pool(name="ps4", bufs=2, space="PSUM") as PS4, \
                 tc.tile_pool(name="sb4", bufs=3) as SB4:
                for t in range(NT):
                    acc = PS4.tile([128, C], f32, tag="pj", name="acc")
                    for c in range(NC_):
                        ycol = yT[c][:, 128 * t:128 * (t + 1)]
                        nc.tensor.matmul(acc[:, 0:512], ycol,
                                         wpt[c][:, 0:512],
                                         start=(c == 0), stop=(c == NC_ - 1))
                        nc.tensor.matmul(acc[:, 512:C], ycol,
                                         wpt[c][:, 512:C],
                                         start=(c == 0), stop=(c == NC_ - 1))
                    ot = SB4.tile([128, C], f32, tag="ot", bufs=3, name="ot")
                    nc.vector.tensor_tensor(ot[:], acc[:], bpB[:],
                                            mybir.AluOpType.add)
                    nc.sync.dma_start(out=out_d[128 * t:128 * (t + 1), :],
                                      in_=ot[:])

    return nc


_WAIT_SKIP = {"InstNoOp", "InstEventSemOp", "InstSemaphoreOp",
              "InstPartitionBroadcast", "InstPartitionAllReduce"}


def _legalize_waits(nc):
    """walrus's codegen allows limited sync-wait commands per ISA struct
    (e.g. a Matmult's waits all land on the generated LDWEIGHTS struct which
    has one slot). Move excess waits onto same-engine NoOps inserted
    immediately before the instruction - program order on the engine queue
    preserves the synchronization semantics."""
    nfix = 0
    for fn in nc.m.functions:
        for bb in fn.blocks:
            out = []
            for ins in bb.instructions:
                si = ins.sync_info
                if (type(ins).__name__ not in _WAIT_SKIP and si is not None
                        and si.on_wait and len(si.on_wait) > 1):
                    waits = list(si.on_wait)
                    extra, keep = waits[:-1], waits[-1:]
                    for k, w in enumerate(extra):
                        nop = mybir.InstNoOp(name=f"{ins.name}-wf{k}", ins=[],
                                             outs=[])
                        nop.engine = ins.engine
                        nop.sync_info = mybir.SyncInfo(on_wait=[w],
                                                       on_update=[])
                        out.append(nop)
                    ins.sync_info = mybir.SyncInfo(
                        on_wait=keep, on_update=list(si.on_update or []))
                    nfix += 1
                out.append(ins)
            bb.instructions = out
    return nfix


_cached_module = None


def _get_module():
    global _cached_module
    if _cached_module is None:
        nc = build_module()
        # populate .instr bytes for InstCustomDveAnt (reciprocal_approx_fast)
        # - Bacc.compile() runs this pass but the raw-Bass path doesn't, and
        # walrus codegen fails with "ISA wrong length" on empty .instr
        mybir.codegen_inst_isa_subclasses(nc)
        _legalize_waits(nc)
        _cached_module = nc
    return _cached_module


def make_in_maps(x, W_attn, b_attn, W_proj, b_proj):
    import ml_dtypes
    bf = ml_dtypes.bfloat16
    x = np.asarray(x, dtype=np.float32)
    wa = np.asarray(W_attn, dtype=np.float32)
    wqk = np.ascontiguousarray(wa[:, 0:2 * C].astype(bf))
    wv = np.ascontiguousarray(wa[:, 2 * C:3 * C].astype(bf))
    wp = np.ascontiguousarray(np.asarray(W_proj, dtype=np.float32).astype(bf))
    ba = np.ascontiguousarray(
        np.asarray(b_attn, dtype=np.float32).reshape(1, C3))
    bp = np.ascontiguousarray(
        np.asarray(b_proj, dtype=np.float32).reshape(1, C))
    return [
        dict(xT=np.ascontiguousarray(x[b].T.astype(bf)),
             Wqk=wqk, Wv=wv, Wp=wp, b_attn=ba, b_proj=bp)
        for b in range(x.shape[0])
    ]


def run(x, W_attn, b_attn, W_proj, b_proj, trace=False, **spmd_kwargs):
    nc = _get_module()
    in_maps = make_in_maps(x, W_attn, b_attn, W_proj, b_proj)
    res = run_bass_kernel_spmd(nc, in_maps, list(range(NCORES)), trace=trace,
                               **spmd_kwargs)
    out = np.stack([res.results[b]["out"] for b in range(len(in_maps))],
                   axis=0)
    return out, res


def kernel(x, W_attn, b_attn, W_proj, b_proj):
    out, _ = run(x, W_attn, b_attn, W_proj, b_proj)
    return out


# revision 39
# speedup vs baseline: 1.1401x; 1.1401x over previous
"""Causal self-attention Trainium2 Bass kernel (V5).

Full-input contract: kernel(**inputs) takes the unsharded inputs
(x [8,1024,768], W_attn [768,2304], b_attn [2304], W_proj [768,768],
b_proj [768]) and returns the full output [8,1024,768].

Sharding: data parallel - batch element b runs on NeuronCore b (B=8 =
n_cores), no collectives needed.

V5 changes vs V4 (trace-driven; V4 331us, PE 60% cov, HAM throttled to
K=4/8 for the entire 188us attention phase, 40us DVE reciprocal, 88us
ACT exp):
  - host-side prep: x is transposed and cast to bf16 on the host
    (xT input [768,1024]); W_attn split into Wq/Wk/Wv and cast bf16;
    W_proj bf16. Kills the 48 PE transposes + DVE copies of phase 1 and
    halves weight DMA bytes.
  - all GEMMs run on bf16 operands (fp32 PSUM accumulation).
  - attention pipeline unit is a (k-tile, col-half) CHUNK with fp32
    scores in a [128, 2x512] PSUM tile (2 banks, both heads of the
    pair). Chunks are double-buffered (4 banks) next to the 4 avp
    banks, so the score MM for chunk n+2 no longer waits on exp(n):
    the PE never idles long enough for HAM to re-throttle.
  - one exp per chunk covers both heads ([128, 2, n] AP) - halves ACT
    instruction count; one affine_select masks both heads' diagonal.
  - softmax divide: l-rows gathered by SBUF->SBUF DMA into a [4,512]
    tile per head pair, ONE reciprocal_approx_fast (the V4 kernel spent
    40us in 12 full-precision Newton reciprocals), DMA broadcast,
    DVE multiply fused into yT (bf16).
"""

import os
import sys

import numpy as np

for _p in ("/opt/trn_rl_repo", "/root/.axon_site/_ro/trn_rl_repo"):
    if os.path.isdir(_p) and _p not in sys.path:
        sys.path.insert(0, _p)
        break

import concourse.bass as bass
import concourse.mybir as mybir
import concourse.tile as tile
from concourse.bass_utils import run_bass_kernel_spmd

T, C, H = 1024, 768, 12
C3 = 3 * C
NCORES = 8
NT = T // 128    # 8 t-tiles
NC_ = C // 128   # 6 c-tiles
NHP = H // 2     # 6 head pairs
f32 = mybir.dt.float32
bf16 = mybir.dt.bfloat16

EXP = mybir.ActivationFunctionType.Exp


def build_module():
    nc = bass.Bass()
    xT_d = nc.dram_tensor("xT", [C, T], bf16, kind="ExternalInput")
    wqk_d = nc.dram_tensor("Wqk", [C, 2 * C], bf16, kind="ExternalInput")
    wv_d = nc.dram_tensor("Wv", [C, C], bf16, kind="ExternalInput")
    wp_d = nc.dram_tensor("Wp", [C, C], bf16, kind="ExternalInput")
    ba_d = nc.dram_tensor("b_attn", [1, C3], f32, kind="ExternalInput")
    bp_d = nc.dram_tensor("b_proj", [1, C], f32, kind="ExternalInput")
    out_d = nc.dram_tensor("out", [T, C], f32, kind="ExternalOutput")

    with tile.TileContext(nc) as tc:
        with tc.tile_pool(name="persist", bufs=1) as P0:
            qkT = [P0.tile([128, T], bf16, name=f"qkT{m}") for m in range(2 * NC_)]
            # per head: 64 v-dim columns + 64 ones-columns. The AV matmul
            # then emits y rows at partitions 0:64 AND the softmax
            # denominator l replicated across partitions 64:128 - a free
            # partition-broadcast on the PE (MM cost depends only on the
            # moving-operand columns).
            vA = [P0.tile([128, 128 * H], bf16, name=f"vA{t}") for t in range(NT)]
            yT = [P0.tile([128, T], bf16, name=f"yT{c}") for c in range(NC_)]
            ba_sb = P0.tile([1, C], f32, name="ba_sb")
            bp_sb = P0.tile([1, C], f32, name="bp_sb")
            baB = P0.tile([128, C], f32, name="baB")   # b_attn v-part bcast
            bpB = P0.tile([128, C], f32, name="bpB")   # b_proj bcast
            wpt = [P0.tile([128, C], bf16, name=f"wp{c}") for c in range(NC_)]
            bqk = [P0.tile([128, 1], f32, name=f"bqk{m}") for m in range(2 * NC_)]
            # xT / q|k weights persist into the attention phase: the q^T/k^T
            # GEMM for head pair hp+1 is interleaved into hp's attention
            xT = [P0.tile([128, T], bf16, name=f"xT{c}") for c in range(NC_)]
            wQK = [P0.tile([128, 2 * C], bf16, name=f"wQK{c}")
                   for c in range(NC_)]
            warm_src = P0.tile([1, 16], f32, name="warm_src")
            nc.vector.memset(warm_src[:], 1.0)

            # preload the exp table while ACT is idle (else the first
            # attention exp pays the ~2.7us ACT_TABLE_LOAD inline)
            warm = P0.tile([1, 16], f32, name="warm")
            nc.scalar.activation(warm[:], warm_src[:], EXP, scale=0.125)

            # ---- phase A: v GEMM (x arrives pre-transposed bf16) ----
            with tc.tile_pool(name="sbA", bufs=1) as SBA:
                wV = [SBA.tile([128, C], bf16, name=f"wV{c}", tag=f"wV{c}",
                               bufs=1) for c in range(NC_)]
                # interleave x/weight loads across both HWDGE queues so the
                # first v-GEMM accumulation chain can start ~2 tiles in
                nc.sync.dma_start(out=ba_sb[:], in_=ba_d[0:1, 2 * C:3 * C])
                for c in range(NC_):
                    q = nc.sync if c % 2 == 0 else nc.scalar
                    q2 = nc.scalar if c % 2 == 0 else nc.sync
                    q.dma_start(out=xT[c][:],
                                in_=xT_d[128 * c:128 * (c + 1), :])
                    q2.dma_start(out=wV[c][:],
                                 in_=wv_d[128 * c:128 * (c + 1), :])
                # one-time bias broadcast (free-dim stride-0 DMA replicate)
                # on the gpsimd SWDGE queue (descriptor gen on Q7, off both
                # HWDGE queues)
                nc.gpsimd.dma_start(
                    out=baB[:],
                    in_=ba_sb[0:1, :].unsqueeze(1).to_broadcast([1, 128, C]))
                baB_r = baB.rearrange("p (h e) -> p h e", h=H)
                # bqk partition-scatter DMAs (4B-granular, slow to issue) on
                # the gpsimd SWDGE queue, off the weight-load path
                for m in range(2 * NC_):
                    nc.gpsimd.dma_start(
                        out=bqk[m][:],
                        in_=ba_d[0:1, 128 * m:128 * (m + 1)]
                            .rearrange("a p -> p a"))
                # q|k weight loads stream behind the v weights; two DMAs per
                # tile - a single [128,1536] load (3KB rows) hits a slow
                # descriptor path (>10us issue), 1.5KB rows issue in ~0.6us
                for c in range(NC_):
                    q = nc.sync if c % 2 == 0 else nc.scalar
                    q2 = nc.scalar if c % 2 == 0 else nc.sync
                    q.dma_start(out=wQK[c][:, 0:C],
                                in_=wqk_d[128 * c:128 * (c + 1), 0:C])
                    q2.dma_start(out=wQK[c][:, C:2 * C],
                                 in_=wqk_d[128 * c:128 * (c + 1), C:2 * C])

                with tc.tile_pool(name="psA", bufs=1, space="PSUM") as PSA:
                    # v: stationary xT columns, moving W_v rows. bufs=4 (the
                    # whole PSUM): every t-tile needs ALL SIX xT/wV tiles, so
                    # until the last weight DMA lands only partial-product
                    # chains can run - 4 accumulators let 4 t-tiles' partials
                    # proceed DMA-paced instead of serializing at the end.
                    for t in range(NT):
                        accv = PSA.tile([128, C], f32, tag="v", bufs=4,
                                        name="accv")
                        for c in range(NC_):
                            xcol = xT[c][:, 128 * t:128 * (t + 1)]
                            nc.tensor.matmul(accv[:, 0:512], xcol,
                                             wV[c][:, 0:512],
                                             start=(c == 0), stop=(c == NC_ - 1))
                            nc.tensor.matmul(accv[:, 512:C], xcol,
                                             wV[c][:, 512:C],
                                             start=(c == 0), stop=(c == NC_ - 1))
                        # per-head layout [ones(64) | v(64)]: the ones FIRST
                        # so the AV matmul puts the replicated l at
                        # partitions 0:64 - the custom-DVE reciprocal ignores
                        # a shifted input partition base, standard TT doesn't
                        av = vA[t].rearrange("p (h e) -> p h e", h=H)
                        nc.vector.memset(av[:, :, 0:64], 1.0)
                        # eviction with fused bias add
                        nc.vector.tensor_tensor(
                            av[:, :, 64:128],
                            accv[:].rearrange("p (h e) -> p h e", h=H),
                            baB_r[:, :, 0:64],
                            mybir.AluOpType.add)

            # ---- phase B: attention with interleaved q^T/k^T GEMMs ----
            with tc.tile_pool(name="ps3", bufs=1, space="PSUM") as PS3, \
                 tc.tile_pool(name="sb3", bufs=1) as SB3:
                from collections import deque
                pending = deque()   # deferred normalization pipeline stages

                def pop_pending(k=2):
                    n = 0
                    while pending and n < k:
                        s = pending.popleft()
                        if s is not None:
                            s()
                        n += 1

                # chunk list: (i, w); w=0 -> query cols [lo,512) (i<4 only),
                # w=1 -> [max(lo,512), 1024). L chunks first so the L-half
                # finishes early - its normalization frees the avs L banks
                # for the interleaved qk jobs with slack to spare.
                chunks = [(i, 0) for i in range(4)] + \
                         [(i, 1) for i in range(NT)]

                def chunk_cols(i, w):
                    lo = 128 * i
                    if w == 0:
                        return lo, 512
                    return max(lo, 512), T

                def emit_qk_job(tp, jidx, tags):
                    # one [128,512] slice of q^T (jidx 0/1) or k^T (2/3) for
                    # target head pair tp; the accumulator borrows an
                    # avs-tagged PSUM bank (free between L-normalization and
                    # the next pair's AV allocation)
                    m = tp if jidx < 2 else NC_ + tp
                    j2 = jidx % 2
                    acc = PS3.tile([128, 512], f32, tag=tags[jidx], bufs=1,
                                   name="qka")
                    for c in range(NC_):
                        nc.tensor.matmul(
                            acc[:], wQK[c][:, 128 * m:128 * (m + 1)],
                            xT[c][:, 512 * j2:512 * (j2 + 1)],
                            start=(c == 0), stop=(c == NC_ - 1))
                    # psum -> sbuf(bf16) with per-partition bias add
                    nc.vector.tensor_scalar_add(
                        qkT[m][:, 512 * j2:512 * (j2 + 1)], acc[:],
                        bqk[m][:])

                sps = {}    # (hp, ch) -> score PSUM tile
                pbs = {}    # (hp, ch) -> exp'd SBUF tile
                avst = {}   # (hp, hs, half) -> [128,512] accumulator

                def emit_score(hp, ch):
                    i, w = ch
                    lo = 128 * i
                    c0, c1 = chunk_cols(i, w)
                    qt = qkT[hp]
                    kt = qkT[NC_ + hp]
                    scp = PS3.tile([128, 1024], f32, tag="sc", bufs=2,
                                   name="scp")
                    for hs in range(2):
                        base = 64 * hs
                        nc.tensor.matmul(
                            scp[:, 512 * hs:512 * hs + (c1 - c0)],
                            kt[base:base + 64, lo:lo + 128],
                            qt[base:base + 64, c0:c1],
                            start=True, stop=True)
                    sps[(hp, ch)] = scp

                def emit_exp(hp, ch):
                    i, w = ch
                    lo = 128 * i
                    c0, c1 = chunk_cols(i, w)
                    n = c1 - c0
                    scp = sps.pop((hp, ch))
                    pb = SB3.tile([128, 1024], bf16, tag="pb", bufs=4,
                                  name="pb")
                    scv = scp.rearrange("p (s n) -> p s n", s=2)
                    pbv = pb.rearrange("p (s n) -> p s n", s=2)
                    nc.scalar.activation(pbv[:, :, 0:n], scv[:, :, 0:n],
                                         EXP, scale=0.125)
                    if c0 == lo:
                        # diagonal [128,128] block (both heads):
                        # keep iff q - key >= 0
                        nc.gpsimd.affine_select(
                            out=pbv[:, :, 0:128], in_=pbv[:, :, 0:128],
                            pattern=[[0, 2], [1, 128]],
                            compare_op=mybir.AluOpType.is_ge, fill=0.0,
                            base=0, channel_multiplier=-1,
                        )
                    pbs[(hp, ch)] = pb

                def emit_av(hp, ch):
                    i, w = ch
                    c0, c1 = chunk_cols(i, w)
                    n = c1 - c0
                    pb = pbs.pop((hp, ch))
                    if i == 0 and w == 0:
                        for hs in range(2):
                            for half in range(2):
                                avst[(hp, hs, half)] = PS3.tile(
                                    [128, 512], f32, tag=f"av{hs}{half}",
                                    bufs=1, name=f"av{hs}{half}")
                    for hs in range(2):
                        h = 2 * hp + hs
                        vt = vA[i][:, 128 * h:128 * h + 128]
                        if w == 0:
                            nc.tensor.matmul(
                                avst[(hp, hs, 0)][:, c0:512], vt,
                                pb[:, 512 * hs:512 * hs + n],
                                start=(i == 0), stop=(i == 3),
                                skip_group_check=True)
                        else:
                            nc.tensor.matmul(
                                avst[(hp, hs, 1)][:, c0 - 512:512], vt,
                                pb[:, 512 * hs:512 * hs + n],
                                start=(i == 0), stop=(i == NT - 1),
                                skip_group_check=True)

                def make_norm(hp, half):
                    # avs rows 0:64 hold l replicated across partitions
                    # (ones-columns in vA), rows 64:128 hold y.
                    # reciprocal + normalize read PSUM directly - no
                    # staging copies, no DMA gathers/broadcasts.
                    loc = {}

                    def s_recip():
                        for hs in range(2):
                            rli = SB3.tile([64, 512], f32,
                                           tag=f"rli{hs}{half}", bufs=2,
                                           name=f"rli{hs}{half}")
                            nc.vector.reciprocal_approx_fast(
                                rli[:],
                                avst[(hp, hs, half)][0:64, 0:512])
                            loc[hs] = rli

                    def s_mult():
                        for hs in range(2):
                            base = 64 * hs
                            nc.vector.tensor_tensor(
                                yT[hp][base:base + 64,
                                       512 * half:512 * (half + 1)],
                                avst[(hp, hs, half)][64:128, 0:512],
                                loc[hs][:],
                                mybir.AluOpType.mult)

                    return [s_recip, s_mult]

                # prelude for head pair 0: the L-half chunks only need the
                # j2=0 (query/key cols < 512) jobs, so those go first and
                # the first score/exp pairs interleave with jobs 1 and 3
                JTAGS = ["av00", "av10", "av00", "av10"]
                nch = len(chunks)
                emit_qk_job(0, 0, ["av00"] * 4)
                emit_qk_job(0, 2, ["av10"] * 4)
                emit_score(0, chunks[0])
                emit_exp(0, chunks[0])
                emit_qk_job(0, 1, ["av01"] * 4)
                emit_score(0, chunks[1])
                emit_exp(0, chunks[1])
                emit_qk_job(0, 3, ["av11"] * 4)

                for hp in range(NHP):
                    if hp == 0:
                        # W_proj / b_proj loads: sync-queue only (a scalar-
                        # queue DMA here would head-block the exp stream)
                        nc.sync.dma_start(out=bp_sb[:], in_=bp_d[:])
                        for c in range(NC_):
                            nc.sync.dma_start(
                                out=wpt[c][:],
                                in_=wp_d[128 * c:128 * (c + 1), :])
                        nc.gpsimd.dma_start(
                            out=bpB[:],
                            in_=bp_sb[0:1, :].unsqueeze(1)
                                .to_broadcast([1, 128, C]))
                    # steady state: scores TWO chunks ahead of the AV
                    # stream so the in-order PE queue always has a score MM
                    # to run while AV(n) waits on exp/affine(n). The first
                    # two score/exp pairs were emitted in the previous
                    # pair's tail (or the prelude).
                    for n_ in range(2, nch):
                        pop_pending()
                        emit_score(hp, chunks[n_])
                        emit_exp(hp, chunks[n_])
                        emit_av(hp, chunks[n_ - 2])
                        if n_ == 5:
                            # L-half normalization inline: the avs L banks
                            # are re-tagged as qk accumulators right after
                            for s in make_norm(hp, 0):
                                s()
                        elif n_ in (6, 8, 10) and hp + 1 < NHP:
                            emit_qk_job(hp + 1, {6: 0, 8: 1, 10: 2}[n_],
                                        JTAGS)
                    pop_pending()
                    emit_av(hp, chunks[nch - 2])
                    emit_av(hp, chunks[nch - 1])
                    if hp + 1 < NHP:
                        # job3 first: its eviction gates the next pair's
                        # avs allocation (same av10 tag), so give it the
                        # warmup's worth of slack
                        emit_qk_job(hp + 1, 3, JTAGS)
                        # next pair's warmup in this tail: keeps ACT fed
                        # across the pair boundary (its inputs - the j2=0
                        # qk jobs - completed mid-pair)
                        emit_score(hp + 1, chunks[0])
                        emit_exp(hp + 1, chunks[0])
                        emit_score(hp + 1, chunks[1])
                        emit_exp(hp + 1, chunks[1])
                    pending.extend(make_norm(hp, 1))

                # drain the tail (last pair's normalization)
                while pending:
                    s = pending.popleft()
                    if s is not None:
                        s()

            # ---- phase C: out = y^T.T @ W_proj + b_proj ----
            with tc.tile_pool(name="ps4", bufs=2, space="PSUM") as PS4, \
                 tc.tile_pool(name="sb4", bufs=3) as SB4:
                for t in range(NT):
                    acc = PS4.tile([128, C], f32, tag="pj", name="acc")
                    for c in range(NC_):
                        ycol = yT[c][:, 128 * t:128 * (t + 1)]
                        nc.tensor.matmul(acc[:, 0:512], ycol,
                                         wpt[c][:, 0:512],
                                         start=(c == 0), stop=(c == NC_ - 1))
                        nc.tensor.matmul(acc[:, 512:C], ycol,
                                         wpt[c][:, 512:C],
                                         start=(c == 0), stop=(c == NC_ - 1))
                    ot = SB4.tile([128, C], f32, tag="ot", bufs=3, name="ot")
                    nc.vector.tensor_tensor(ot[:], acc[:], bpB[:],
                                            mybir.AluOpType.add)
                    nc.sync.dma_start(out=out_d[128 * t:128 * (t + 1), :],
                                      in_=ot[:])

    return nc


_WAIT_SKIP = {"InstNoOp", "InstEventSemOp", "InstSemaphoreOp",
              "InstPartitionBroadcast", "InstPartitionAllReduce"}


def _legalize_waits(nc):
    """walrus's codegen allows limited sync-wait commands per ISA struct
    (e.g. a Matmult's waits all land on the generated LDWEIGHTS struct which
    has one slot). Move excess waits onto same-engine NoOps inserted
    immediately before the instruction - program order on the engine queue
    preserves the synchronization semantics."""
    nfix = 0
    for fn in nc.m.functions:
        for bb in fn.blocks:
            out = []
            for ins in bb.instructions:
                si = ins.sync_info
                if (type(ins).__name__ not in _WAIT_SKIP and si is not None
                        and si.on_wait and len(si.on_wait) > 1):
                    waits = list(si.on_wait)
                    extra, keep = waits[:-1], waits[-1:]
                    for k, w in enumerate(extra):
                        nop = mybir.InstNoOp(name=f"{ins.name}-wf{k}", ins=[],
                                             outs=[])
                        nop.engine = ins.engine
                        nop.sync_info = mybir.SyncInfo(on_wait=[w],
                                                       on_update=[])
                        out.append(nop)
                    ins.sync_info = mybir.SyncInfo(
                        on_wait=keep, on_update=list(si.on_update or []))
                    nfix += 1
                out.append(ins)
            bb.instructions = out
    return nfix


_cached_module = None


def _get_module():
    global _cached_module
    if _cached_module is None:
        nc = build_module()
        # populate .instr bytes for InstCustomDveAnt (reciprocal_approx_fast)
        # - Bacc.compile() runs this pass but the raw-Bass path doesn't, and
        # walrus codegen fails with "ISA wrong length" on empty .instr
        mybir.codegen_inst_isa_subclasses(nc)
        _legalize_waits(nc)
        _cached_module = nc
    return _cached_module


def make_in_maps(x, W_attn, b_attn, W_proj, b_proj):
    import ml_dtypes
    bf = ml_dtypes.bfloat16
    x = np.asarray(x, dtype=np.float32)
    wa = np.asarray(W_attn, dtype=np.float32)
    wqk = np.ascontiguousarray(wa[:, 0:2 * C].astype(bf))
    wv = np.ascontiguousarray(wa[:, 2 * C:3 * C].astype(bf))
    wp = np.ascontiguousarray(np.asarray(W_proj, dtype=np.float32).astype(bf))
    ba = np.ascontiguousarray(
        np.asarray(b_attn, dtype=np.float32).reshape(1, C3))
    bp = np.ascontiguousarray(
        np.asarray(b_proj, dtype=np.float32).reshape(1, C))
    return [
        dict(xT=np.ascontiguousarray(x[b].T.astype(bf)),
             Wqk=wqk, Wv=wv, Wp=wp, b_attn=ba, b_proj=bp)
        for b in range(x.shape[0])
    ]


def run(x, W_attn, b_attn, W_proj, b_proj, trace=False, **spmd_kwargs):
    nc = _get_module()
    in_maps = make_in_maps(x, W_attn, b_attn, W_proj, b_proj)
    res = run_bass_kernel_spmd(nc, in_maps, list(range(NCORES)), trace=trace,
                               **spmd_kwargs)
    out = np.stack([res.results[b]["out"] for b in range(len(in_maps))],
                   axis=0)
    return out, res


def kernel(x, W_attn, b_attn, W_proj, b_proj):
    out, _ = run(x, W_attn, b_attn, W_proj, b_proj)
    return out


# revision 42
# speedup vs baseline: 1.1463x; 1.0054x over previous
"""Causal self-attention Trainium2 Bass kernel (V5).

Full-input contract: kernel(**inputs) takes the unsharded inputs
(x [8,1024,768], W_attn [768,2304], b_attn [2304], W_proj [768,768],
b_proj [768]) and returns the full output [8,1024,768].

Sharding: data parallel - batch element b runs on NeuronCore b (B=8 =
n_cores), no collectives needed.

V5 changes vs V4 (trace-driven; V4 331us, PE 60% cov, HAM throttled to
K=4/8 for the entire 188us attention phase, 40us DVE reciprocal, 88us
ACT exp):
  - host-side prep: x is transposed and cast to bf16 on the host
    (xT input [768,1024]); W_attn split into Wq/Wk/Wv and cast bf16;
    W_proj bf16. Kills the 48 PE transposes + DVE copies of phase 1 and
    halves weight DMA bytes.
  - all GEMMs run on bf16 operands (fp32 PSUM accumulation).
  - attention pipeline unit is a (k-tile, col-half) CHUNK with fp32
    scores in a [128, 2x512] PSUM tile (2 banks, both heads of the
    pair). Chunks are double-buffered (4 banks) next to the 4 avp
    banks, so the score MM for chunk n+2 no longer waits on exp(n):
    the PE never idles long enough for HAM to re-throttle.
  - one exp per chunk covers both heads ([128, 2, n] AP) - halves ACT
    instruction count; one affine_select masks both heads' diagonal.
  - softmax divide: l-rows gathered by SBUF->SBUF DMA into a [4,512]
    tile per head pair, ONE reciprocal_approx_fast (the V4 kernel spent
    40us in 12 full-precision Newton reciprocals), DMA broadcast,
    DVE multiply fused into yT (bf16).
"""

import os
import sys

import numpy as np

for _p in ("/opt/trn_rl_repo", "/root/.axon_site/_ro/trn_rl_repo"):
    if os.path.isdir(_p) and _p not in sys.path:
        sys.path.insert(0, _p)
        break

import concourse.bass as bass
import concourse.mybir as mybir
import concourse.tile as tile
from concourse.bass_utils import run_bass_kernel_spmd

T, C, H = 1024, 768, 12
C3 = 3 * C
NCORES = 8
NT = T // 128    # 8 t-tiles
NC_ = C // 128   # 6 c-tiles
NHP = H // 2     # 6 head pairs
f32 = mybir.dt.float32
bf16 = mybir.dt.bfloat16

EXP = mybir.ActivationFunctionType.Exp


def build_module():
    nc = bass.Bass()
    xT_d = nc.dram_tensor("xT", [C, T], bf16, kind="ExternalInput")
    wqk_d = nc.dram_tensor("Wqk", [C, 2 * C], bf16, kind="ExternalInput")
    wv_d = nc.dram_tensor("Wv", [C, C], bf16, kind="ExternalInput")
    wp_d = nc.dram_tensor("Wp", [C, C], bf16, kind="ExternalInput")
    ba_d = nc.dram_tensor("b_attn", [1, C3], f32, kind="ExternalInput")
    bp_d = nc.dram_tensor("b_proj", [1, C], f32, kind="ExternalInput")
    out_d = nc.dram_tensor("out", [T, C], f32, kind="ExternalOutput")

    with tile.TileContext(nc) as tc:
        with tc.tile_pool(name="persist", bufs=1) as P0:
            qkT = [P0.tile([128, T], bf16, name=f"qkT{m}") for m in range(2 * NC_)]
            # per head: 64 v-dim columns + 64 ones-columns. The AV matmul
            # then emits y rows at partitions 0:64 AND the softmax
            # denominator l replicated across partitions 64:128 - a free
            # partition-broadcast on the PE (MM cost depends only on the
            # moving-operand columns).
            vA = [P0.tile([128, 128 * H], bf16, name=f"vA{t}") for t in range(NT)]
            yT = [P0.tile([128, T], bf16, name=f"yT{c}") for c in range(NC_)]
            ba_sb = P0.tile([1, C], f32, name="ba_sb")
            bp_sb = P0.tile([1, C], f32, name="bp_sb")
            baB = P0.tile([128, C], f32, name="baB")   # b_attn v-part bcast
            bpB = P0.tile([128, C], f32, name="bpB")   # b_proj bcast
            wpt = [P0.tile([128, C], bf16, name=f"wp{c}") for c in range(NC_)]
            bqk = [P0.tile([128, 1], f32, name=f"bqk{m}") for m in range(2 * NC_)]
            # xT / q|k weights persist into the attention phase: the q^T/k^T
            # GEMM for head pair hp+1 is interleaved into hp's attention
            xT = [P0.tile([128, T], bf16, name=f"xT{c}") for c in range(NC_)]
            wQK = [P0.tile([128, 2 * C], bf16, name=f"wQK{c}")
                   for c in range(NC_)]
            warm_src = P0.tile([1, 16], f32, name="warm_src")
            nc.vector.memset(warm_src[:], 1.0)
            # causal mask for the diagonal [128,128] blocks (both heads):
            # keep iff q - key >= 0. Applied as a DVE multiply (the gpsimd
            # affine_select added an extra engine hop to the exp->AV chain)
            mask2 = P0.tile([128, 256], bf16, name="mask2")
            nc.vector.memset(mask2[:], 1.0)
            m2v = mask2.rearrange("p (s n) -> p s n", s=2)
            nc.gpsimd.affine_select(
                out=m2v[:], in_=m2v[:], pattern=[[0, 2], [1, 128]],
                compare_op=mybir.AluOpType.is_ge, fill=0.0,
                base=0, channel_multiplier=-1,
            )

            # preload the exp table while ACT is idle (else the first
            # attention exp pays the ~2.7us ACT_TABLE_LOAD inline)
            warm = P0.tile([1, 16], f32, name="warm")
            nc.scalar.activation(warm[:], warm_src[:], EXP, scale=0.125)

            # ---- phase A: v GEMM (x arrives pre-transposed bf16) ----
            with tc.tile_pool(name="sbA", bufs=1) as SBA:
                wV = [SBA.tile([128, C], bf16, name=f"wV{c}", tag=f"wV{c}",
                               bufs=1) for c in range(NC_)]
                # interleave x/weight loads across both HWDGE queues so the
                # first v-GEMM accumulation chain can start ~2 tiles in
                nc.sync.dma_start(out=ba_sb[:], in_=ba_d[0:1, 2 * C:3 * C])
                for c in range(NC_):
                    q = nc.sync if c % 2 == 0 else nc.scalar
                    q2 = nc.scalar if c % 2 == 0 else nc.sync
                    q.dma_start(out=xT[c][:],
                                in_=xT_d[128 * c:128 * (c + 1), :])
                    q2.dma_start(out=wV[c][:],
                                 in_=wv_d[128 * c:128 * (c + 1), :])
                # one-time bias broadcast (free-dim stride-0 DMA replicate)
                # on the gpsimd SWDGE queue (descriptor gen on Q7, off both
                # HWDGE queues)
                nc.gpsimd.dma_start(
                    out=baB[:],
                    in_=ba_sb[0:1, :].unsqueeze(1).to_broadcast([1, 128, C]))
                baB_r = baB.rearrange("p (h e) -> p h e", h=H)
                # bqk partition-scatter DMAs (4B-granular, slow to issue) on
                # the gpsimd SWDGE queue, off the weight-load path
                for m in range(2 * NC_):
                    nc.gpsimd.dma_start(
                        out=bqk[m][:],
                        in_=ba_d[0:1, 128 * m:128 * (m + 1)]
                            .rearrange("a p -> p a"))
                # q|k weight loads stream behind the v weights; two DMAs per
                # tile - a single [128,1536] load (3KB rows) hits a slow
                # descriptor path (>10us issue), 1.5KB rows issue in ~0.6us
                for c in range(NC_):
                    q = nc.sync if c % 2 == 0 else nc.scalar
                    q2 = nc.scalar if c % 2 == 0 else nc.sync
                    q.dma_start(out=wQK[c][:, 0:C],
                                in_=wqk_d[128 * c:128 * (c + 1), 0:C])
                    q2.dma_start(out=wQK[c][:, C:2 * C],
                                 in_=wqk_d[128 * c:128 * (c + 1), C:2 * C])

                # per-head layout [ones(64) | v(64)]: the ones FIRST so the
                # AV matmul puts the replicated l at partitions 0:64 - the
                # custom-DVE reciprocal ignores a shifted input partition
                # base, standard TT doesn't. Ones written up front while
                # DVE is idle.
                for t in range(NT):
                    av = vA[t].rearrange("p (h e) -> p h e", h=H)
                    nc.vector.memset(av[:, :, 0:64], 1.0)

                with tc.tile_pool(name="psA", bufs=1, space="PSUM") as PSA:
                    # v GEMM, c-OUTER with 8 single-bank half-accumulators:
                    # every t-tile needs ALL SIX xT/wV tiles, and the PE
                    # queue is in-order - with a t-outer loop one stalled
                    # c-tile DMA blocks ready partial products behind it.
                    # c-outer lets every partial chain run DMA-paced.
                    for vh in range(2):   # v column halves: heads 0:6, 6:12
                        accs = {}
                        for c in range(NC_):
                            for t in range(NT):
                                if c == 0:
                                    accs[t] = PSA.tile([128, 384], f32,
                                                       tag="v", bufs=8,
                                                       name="accv")
                                xcol = xT[c][:, 128 * t:128 * (t + 1)]
                                nc.tensor.matmul(
                                    accs[t][:],
                                    xcol, wV[c][:, 384 * vh:384 * (vh + 1)],
                                    start=(c == 0), stop=(c == NC_ - 1))
                        for t in range(NT):
                            av = vA[t].rearrange("p (h e) -> p h e", h=H)
                            # eviction with fused bias add
                            nc.vector.tensor_tensor(
                                av[:, 6 * vh:6 * (vh + 1), 64:128],
                                accs[t][:].rearrange("p (h e) -> p h e", h=6),
                                baB_r[:, 6 * vh:6 * (vh + 1), 0:64],
                                mybir.AluOpType.add)

            # ---- phase B: attention with interleaved q^T/k^T GEMMs ----
            with tc.tile_pool(name="ps3", bufs=1, space="PSUM") as PS3, \
                 tc.tile_pool(name="sb3", bufs=1) as SB3:
                from collections import deque
                pending = deque()   # deferred normalization pipeline stages

                def pop_pending(k=2):
                    n = 0
                    while pending and n < k:
                        s = pending.popleft()
                        if s is not None:
                            s()
                        n += 1

                # chunk list: (i, w); w=0 -> query cols [lo,512) (i<4 only),
                # w=1 -> [max(lo,512), 1024). L chunks first so the L-half
                # finishes early - its normalization frees the avs L banks
                # for the interleaved qk jobs with slack to spare.
                chunks = [(i, 0) for i in range(4)] + \
                         [(i, 1) for i in range(NT)]

                def chunk_cols(i, w):
                    lo = 128 * i
                    if w == 0:
                        return lo, 512
                    return max(lo, 512), T

                def emit_qk_job(tp, jidx, tags):
                    # one [128,512] slice of q^T (jidx 0/1) or k^T (2/3) for
                    # target head pair tp; the accumulator borrows an
                    # avs-tagged PSUM bank (free between L-normalization and
                    # the next pair's AV allocation)
                    m = tp if jidx < 2 else NC_ + tp
                    j2 = jidx % 2
                    acc = PS3.tile([128, 512], f32, tag=tags[jidx], bufs=1,
                                   name="qka")
                    for c in range(NC_):
                        nc.tensor.matmul(
                            acc[:], wQK[c][:, 128 * m:128 * (m + 1)],
                            xT[c][:, 512 * j2:512 * (j2 + 1)],
                            start=(c == 0), stop=(c == NC_ - 1))
                    # psum -> sbuf(bf16) with per-partition bias add
                    nc.vector.tensor_scalar_add(
                        qkT[m][:, 512 * j2:512 * (j2 + 1)], acc[:],
                        bqk[m][:])

                sps = {}    # (hp, ch) -> score PSUM tile
                pbs = {}    # (hp, ch) -> exp'd SBUF tile
                avst = {}   # (hp, hs, half) -> [128,512] accumulator

                def emit_score(hp, ch):
                    i, w = ch
                    lo = 128 * i
                    c0, c1 = chunk_cols(i, w)
                    qt = qkT[hp]
                    kt = qkT[NC_ + hp]
                    scp = PS3.tile([128, 1024], f32, tag="sc", bufs=2,
                                   name="scp")
                    for hs in range(2):
                        base = 64 * hs
                        nc.tensor.matmul(
                            scp[:, 512 * hs:512 * hs + (c1 - c0)],
                            kt[base:base + 64, lo:lo + 128],
                            qt[base:base + 64, c0:c1],
                            start=True, stop=True)
                    sps[(hp, ch)] = scp

                def emit_exp(hp, ch):
                    i, w = ch
                    lo = 128 * i
                    c0, c1 = chunk_cols(i, w)
                    n = c1 - c0
                    scp = sps.pop((hp, ch))
                    pb = SB3.tile([128, 1024], bf16, tag="pb", bufs=4,
                                  name="pb")
                    scv = scp.rearrange("p (s n) -> p s n", s=2)
                    pbv = pb.rearrange("p (s n) -> p s n", s=2)
                    nc.scalar.activation(pbv[:, :, 0:n], scv[:, :, 0:n],
                                         EXP, scale=0.125)
                    if c0 == lo:
                        # diagonal [128,128] block (both heads): DVE multiply
                        # by the precomputed causal mask
                        nc.vector.tensor_tensor(
                            pbv[:, :, 0:128], pbv[:, :, 0:128],
                            m2v[:], mybir.AluOpType.mult)
                    pbs[(hp, ch)] = pb

                def emit_av(hp, ch):
                    i, w = ch
                    c0, c1 = chunk_cols(i, w)
                    n = c1 - c0
                    pb = pbs.pop((hp, ch))
                    if i == 0 and w == 0:
                        for hs in range(2):
                            for half in range(2):
                                avst[(hp, hs, half)] = PS3.tile(
                                    [128, 512], f32, tag=f"av{hs}{half}",
                                    bufs=1, name=f"av{hs}{half}")
                    for hs in range(2):
                        h = 2 * hp + hs
                        vt = vA[i][:, 128 * h:128 * h + 128]
                        if w == 0:
                            nc.tensor.matmul(
                                avst[(hp, hs, 0)][:, c0:512], vt,
                                pb[:, 512 * hs:512 * hs + n],
                                start=(i == 0), stop=(i == 3),
                                skip_group_check=True)
                        else:
                            nc.tensor.matmul(
                                avst[(hp, hs, 1)][:, c0 - 512:512], vt,
                                pb[:, 512 * hs:512 * hs + n],
                                start=(i == 0), stop=(i == NT - 1),
                                skip_group_check=True)

                def make_norm(hp, half):
                    # avs rows 0:64 hold l replicated across partitions
                    # (ones-columns in vA), rows 64:128 hold y.
                    # reciprocal + normalize read PSUM directly - no
                    # staging copies, no DMA gathers/broadcasts.
                    loc = {}

                    def s_recip():
                        for hs in range(2):
                            rli = SB3.tile([64, 512], f32,
                                           tag=f"rli{hs}{half}", bufs=2,
                                           name=f"rli{hs}{half}")
                            nc.vector.reciprocal_approx_fast(
                                rli[:],
                                avst[(hp, hs, half)][0:64, 0:512])
                            loc[hs] = rli

                    def s_mult():
                        for hs in range(2):
                            base = 64 * hs
                            nc.vector.tensor_tensor(
                                yT[hp][base:base + 64,
                                       512 * half:512 * (half + 1)],
                                avst[(hp, hs, half)][64:128, 0:512],
                                loc[hs][:],
                                mybir.AluOpType.mult)

                    return [s_recip, s_mult]

                # prelude for head pair 0: the L-half chunks only need the
                # j2=0 (query/key cols < 512) jobs, so those go first and
                # the first score/exp pairs interleave with jobs 1 and 3
                JTAGS = ["av00", "av10", "av00", "av10"]
                nch = len(chunks)
                emit_qk_job(0, 0, ["av00"] * 4)
                emit_qk_job(0, 2, ["av10"] * 4)
                emit_score(0, chunks[0])
                emit_exp(0, chunks[0])
                emit_qk_job(0, 1, ["av01"] * 4)
                emit_score(0, chunks[1])
                emit_exp(0, chunks[1])
                emit_qk_job(0, 3, ["av11"] * 4)

                for hp in range(NHP):
                    if hp == 0:
                        # W_proj / b_proj loads: sync-queue only (a scalar-
                        # queue DMA here would head-block the exp stream)
                        nc.sync.dma_start(out=bp_sb[:], in_=bp_d[:])
                        for c in range(NC_):
                            nc.sync.dma_start(
                                out=wpt[c][:],
                                in_=wp_d[128 * c:128 * (c + 1), :])
                        nc.gpsimd.dma_start(
                            out=bpB[:],
                            in_=bp_sb[0:1, :].unsqueeze(1)
                                .to_broadcast([1, 128, C]))
                    # steady state: scores TWO chunks ahead of the AV
                    # stream so the in-order PE queue always has a score MM
                    # to run while AV(n) waits on exp/affine(n). The first
                    # two score/exp pairs were emitted in the previous
                    # pair's tail (or the prelude).
                    for n_ in range(2, nch):
                        pop_pending()
                        emit_score(hp, chunks[n_])
                        emit_exp(hp, chunks[n_])
                        emit_av(hp, chunks[n_ - 2])
                        if n_ == 5:
                            # L-half normalization inline: the avs L banks
                            # are re-tagged as qk accumulators right after
                            for s in make_norm(hp, 0):
                                s()
                        elif n_ in (6, 8, 10) and hp + 1 < NHP:
                            emit_qk_job(hp + 1, {6: 0, 8: 1, 10: 2}[n_],
                                        JTAGS)
                    pop_pending()
                    emit_av(hp, chunks[nch - 2])
                    emit_av(hp, chunks[nch - 1])
                    if hp + 1 < NHP:
                        # job3 first: its eviction gates the next pair's
                        # avs allocation (same av10 tag), so give it the
                        # warmup's worth of slack
                        emit_qk_job(hp + 1, 3, JTAGS)
                        # next pair's warmup in this tail: keeps ACT fed
                        # across the pair boundary (its inputs - the j2=0
                        # qk jobs - completed mid-pair)
                        emit_score(hp + 1, chunks[0])
                        emit_exp(hp + 1, chunks[0])
                        emit_score(hp + 1, chunks[1])
                        emit_exp(hp + 1, chunks[1])
                    pending.extend(make_norm(hp, 1))

                # drain the tail (last pair's normalization)
                while pending:
                    s = pending.popleft()
                    if s is not None:
                        s()

            # ---- phase C: out = y^T.T @ W_proj + b_proj ----
            with tc.tile_pool(name="ps4", bufs=2, space="PSUM") as PS4, \
                 tc.tile_pool(name="sb4", bufs=3) as SB4:
                for t in range(NT):
                    acc = PS4.tile([128, C], f32, tag="pj", name="acc")
                    for c in range(NC_):
                        ycol = yT[c][:, 128 * t:128 * (t + 1)]
                        nc.tensor.matmul(acc[:, 0:512], ycol,
                                         wpt[c][:, 0:512],
                                         start=(c == 0), stop=(c == NC_ - 1))
                        nc.tensor.matmul(acc[:, 512:C], ycol,
                                         wpt[c][:, 512:C],
                                         start=(c == 0), stop=(c == NC_ - 1))
                    ot = SB4.tile([128, C], f32, tag="ot", bufs=3, name="ot")
                    nc.vector.tensor_tensor(ot[:], acc[:], bpB[:],
                                            mybir.AluOpType.add)
                    nc.sync.dma_start(out=out_d[128 * t:128 * (t + 1), :],
                                      in_=ot[:])

    return nc


_WAIT_SKIP = {"InstNoOp", "InstEventSemOp", "InstSemaphoreOp",
              "InstPartitionBroadcast", "InstPartitionAllReduce"}


def _legalize_waits(nc):
    """walrus's codegen allows limited sync-wait commands per ISA struct
    (e.g. a Matmult's waits all land on the generated LDWEIGHTS struct which
    has one slot). Move excess waits onto same-engine NoOps inserted
    immediately before the instruction - program order on the engine queue
    preserves the synchronization semantics."""
    nfix = 0
    for fn in nc.m.functions:
        for bb in fn.blocks:
            out = []
            for ins in bb.instructions:
                si = ins.sync_info
                if (type(ins).__name__ not in _WAIT_SKIP and si is not None
                        and si.on_wait and len(si.on_wait) > 1):
                    waits = list(si.on_wait)
                    extra, keep = waits[:-1], waits[-1:]
                    for k, w in enumerate(extra):
                        nop = mybir.InstNoOp(name=f"{ins.name}-wf{k}", ins=[],
                                             outs=[])
                        nop.engine = ins.engine
                        nop.sync_info = mybir.SyncInfo(on_wait=[w],
                                                       on_update=[])
                        out.append(nop)
                    ins.sync_info = mybir.SyncInfo(
                        on_wait=keep, on_update=list(si.on_update or []))
                    nfix += 1
                out.append(ins)
            bb.instructions = out
    return nfix


_cached_module = None


def _get_module():
    global _cached_module
    if _cached_module is None:
        nc = build_module()
        # populate .instr bytes for InstCustomDveAnt (reciprocal_approx_fast)
        # - Bacc.compile() runs this pass but the raw-Bass path doesn't, and
        # walrus codegen fails with "ISA wrong length" on empty .instr
        mybir.codegen_inst_isa_subclasses(nc)
        _legalize_waits(nc)
        _cached_module = nc
    return _cached_module


def make_in_maps(x, W_attn, b_attn, W_proj, b_proj):
    import ml_dtypes
    bf = ml_dtypes.bfloat16
    x = np.asarray(x, dtype=np.float32)
    wa = np.asarray(W_attn, dtype=np.float32)
    wqk = np.ascontiguousarray(wa[:, 0:2 * C].astype(bf))
    wv = np.ascontiguousarray(wa[:, 2 * C:3 * C].astype(bf))
    wp = np.ascontiguousarray(np.asarray(W_proj, dtype=np.float32).astype(bf))
    ba = np.ascontiguousarray(
        np.asarray(b_attn, dtype=np.float32).reshape(1, C3))
    bp = np.ascontiguousarray(
        np.asarray(b_proj, dtype=np.float32).reshape(1, C))
    return [
        dict(xT=np.ascontiguousarray(x[b].T.astype(bf)),
             Wqk=wqk, Wv=wv, Wp=wp, b_attn=ba, b_proj=bp)
        for b in range(x.shape[0])
    ]


def run(x, W_attn, b_attn, W_proj, b_proj, trace=False, **spmd_kwargs):
    nc = _get_module()
    in_maps = make_in_maps(x, W_attn, b_attn, W_proj, b_proj)
    res = run_bass_kernel_spmd(nc, in_maps, list(range(NCORES)), trace=trace,
                               **spmd_kwargs)
    out = np.stack([res.results[b]["out"] for b in range(len(in_maps))],
                   axis=0)
    return out, res


def kernel(x, W_attn, b_attn, W_proj, b_proj):
    out, _ = run(x, W_attn, b_attn, W_proj, b_proj)
    return out


# revision 50
# speedup vs baseline: 1.2171x; 1.0617x over previous
"""Causal self-attention Trainium2 Bass kernel (V5).

Full-input contract: kernel(**inputs) takes the unsharded inputs
(x [8,1024,768], W_attn [768,2304], b_attn [2304], W_proj [768,768],
b_proj [768]) and returns the full output [8,1024,768].

Sharding: data parallel - batch element b runs on NeuronCore b (B=8 =
n_cores), no collectives needed.

V5 changes vs V4 (trace-driven; V4 331us, PE 60% cov, HAM throttled to
K=4/8 for the entire 188us attention phase, 40us DVE reciprocal, 88us
ACT exp):
  - host-side prep: x is transposed and cast to bf16 on the host
    (xT input [768,1024]); W_attn split into Wq/Wk/Wv and cast bf16;
    W_proj bf16. Kills the 48 PE transposes + DVE copies of phase 1 and
    halves weight DMA bytes.
  - all GEMMs run on bf16 operands (fp32 PSUM accumulation).
  - attention pipeline unit is a (k-tile, col-half) CHUNK with fp32
    scores in a [128, 2x512] PSUM tile (2 banks, both heads of the
    pair). Chunks are double-buffered (4 banks) next to the 4 avp
    banks, so the score MM for chunk n+2 no longer waits on exp(n):
    the PE never idles long enough for HAM to re-throttle.
  - one exp per chunk covers both heads ([128, 2, n] AP) - halves ACT
    instruction count; one affine_select masks both heads' diagonal.
  - softmax divide: l-rows gathered by SBUF->SBUF DMA into a [4,512]
    tile per head pair, ONE reciprocal_approx_fast (the V4 kernel spent
    40us in 12 full-precision Newton reciprocals), DMA broadcast,
    DVE multiply fused into yT (bf16).
"""

import os
import sys

import numpy as np

for _p in ("/opt/trn_rl_repo", "/root/.axon_site/_ro/trn_rl_repo"):
    if os.path.isdir(_p) and _p not in sys.path:
        sys.path.insert(0, _p)
        break

import concourse.bass as bass
import concourse.mybir as mybir
import concourse.tile as tile
from concourse.bass_utils import run_bass_kernel_spmd

T, C, H = 1024, 768, 12
C3 = 3 * C
NCORES = 8
NT = T // 128    # 8 t-tiles
NC_ = C // 128   # 6 c-tiles
NHP = H // 2     # 6 head pairs
f32 = mybir.dt.float32
bf16 = mybir.dt.bfloat16

EXP = mybir.ActivationFunctionType.Exp


def build_module():
    nc = bass.Bass()
    xT_d = nc.dram_tensor("xT", [C, T], bf16, kind="ExternalInput")
    wqk_d = nc.dram_tensor("Wqk", [C, 2 * C], bf16, kind="ExternalInput")
    wv_d = nc.dram_tensor("Wv", [C, C], bf16, kind="ExternalInput")
    wp_d = nc.dram_tensor("Wp", [C, C], bf16, kind="ExternalInput")
    # only the q/k bias halves reach the device: the v bias passes through
    # softmax exactly (sum of weights is 1) so it and b_proj fold into a
    # constant output row added on the host
    ba_d = nc.dram_tensor("b_qkT", [128, 2 * NC_], f32, kind="ExternalInput")
    out_d = nc.dram_tensor("out", [T, C], f32, kind="ExternalOutput")

    with tile.TileContext(nc) as tc:
        with tc.tile_pool(name="persist", bufs=1) as P0:
            qkT = [P0.tile([128, T], bf16, name=f"qkT{m}") for m in range(2 * NC_)]
            # per head: 64 v-dim columns + 64 ones-columns. The AV matmul
            # then emits y rows at partitions 0:64 AND the softmax
            # denominator l replicated across partitions 64:128 - a free
            # partition-broadcast on the PE (MM cost depends only on the
            # moving-operand columns).
            vA = [P0.tile([128, 128 * H], bf16, name=f"vA{t}") for t in range(NT)]
            yT = [P0.tile([128, T], bf16, name=f"yT{c}") for c in range(NC_)]
            bqkt = P0.tile([128, 2 * NC_], f32, name="bqkt")
            wpt = [P0.tile([128, C], bf16, name=f"wp{c}") for c in range(NC_)]
            # xT / q|k weights persist into the attention phase: the q^T/k^T
            # GEMM for head pair hp+1 is interleaved into hp's attention
            xT = [P0.tile([128, T], bf16, name=f"xT{c}") for c in range(NC_)]
            wQK = [P0.tile([128, 2 * C], bf16, name=f"wQK{c}")
                   for c in range(NC_)]
            warm_src = P0.tile([1, 16], f32, name="warm_src")
            nc.vector.memset(warm_src[:], 1.0)
            # causal mask for the diagonal [128,128] blocks (both heads):
            # keep iff q - key >= 0. Applied as a DVE multiply (the gpsimd
            # affine_select added an extra engine hop to the exp->AV chain)
            mask2 = P0.tile([128, 256], bf16, name="mask2")
            nc.vector.memset(mask2[:], 1.0)
            m2v = mask2.rearrange("p (s n) -> p s n", s=2)
            nc.gpsimd.affine_select(
                out=m2v[:], in_=m2v[:], pattern=[[0, 2], [1, 128]],
                compare_op=mybir.AluOpType.is_ge, fill=0.0,
                base=0, channel_multiplier=-1,
            )

            # preload the exp table while ACT is idle (else the first
            # attention exp pays the ~2.7us ACT_TABLE_LOAD inline)
            warm = P0.tile([1, 16], f32, name="warm")
            nc.scalar.activation(warm[:], warm_src[:], EXP, scale=0.125)

            # ---- phase A: v GEMM (x arrives pre-transposed bf16) ----
            with tc.tile_pool(name="sbA", bufs=1) as SBA:
                wV = [SBA.tile([128, C], bf16, name=f"wV{c}", tag=f"wV{c}",
                               bufs=1) for c in range(NC_)]
                # interleave x/weight loads across both HWDGE queues so the
                # first v-GEMM accumulation chain can start ~2 tiles in
                nc.sync.dma_start(out=bqkt[:], in_=ba_d[:, :])
                for c in range(NC_):
                    q = nc.sync if c % 2 == 0 else nc.scalar
                    q2 = nc.scalar if c % 2 == 0 else nc.sync
                    q.dma_start(out=xT[c][:],
                                in_=xT_d[128 * c:128 * (c + 1), :])
                    q2.dma_start(out=wV[c][:],
                                 in_=wv_d[128 * c:128 * (c + 1), :])
                # q|k weight loads stream behind the v weights; two DMAs per
                # tile - a single [128,1536] load (3KB rows) hits a slow
                # descriptor path (>10us issue), 1.5KB rows issue in ~0.6us
                for c in range(NC_):
                    q = nc.sync if c % 2 == 0 else nc.scalar
                    q2 = nc.scalar if c % 2 == 0 else nc.sync
                    q.dma_start(out=wQK[c][:, 0:C],
                                in_=wqk_d[128 * c:128 * (c + 1), 0:C])
                    q2.dma_start(out=wQK[c][:, C:2 * C],
                                 in_=wqk_d[128 * c:128 * (c + 1), C:2 * C])

                # per-head layout [ones(64) | v(64)]: the ones FIRST so the
                # AV matmul puts the replicated l at partitions 0:64 - the
                # custom-DVE reciprocal ignores a shifted input partition
                # base, standard TT doesn't. Ones written up front while
                # DVE is idle.
                for t in range(NT):
                    av = vA[t].rearrange("p (h e) -> p h e", h=H)
                    nc.vector.memset(av[:, :, 0:64], 1.0)

                with tc.tile_pool(name="psA", bufs=1, space="PSUM") as PSA:
                    # v GEMM, c-OUTER with 8 single-bank half-accumulators:
                    # every t-tile needs ALL SIX xT/wV tiles, and the PE
                    # queue is in-order - with a t-outer loop one stalled
                    # c-tile DMA blocks ready partial products behind it.
                    # c-outer lets every partial chain run DMA-paced.
                    for vh in range(2):   # v column halves: heads 0:6, 6:12
                        accs = {}
                        for c in range(NC_):
                            for t in range(NT):
                                if c == 0:
                                    accs[t] = PSA.tile([128, 384], f32,
                                                       tag="v", bufs=8,
                                                       name="accv")
                                xcol = xT[c][:, 128 * t:128 * (t + 1)]
                                nc.tensor.matmul(
                                    accs[t][:],
                                    xcol, wV[c][:, 384 * vh:384 * (vh + 1)],
                                    start=(c == 0), stop=(c == NC_ - 1))
                        for t in range(NT):
                            av = vA[t].rearrange("p (h e) -> p h e", h=H)
                            nc.vector.tensor_copy(
                                av[:, 6 * vh:6 * (vh + 1), 64:128],
                                accs[t][:].rearrange("p (h e) -> p h e", h=6))

            # ---- phase B: attention with interleaved q^T/k^T GEMMs ----
            with tc.tile_pool(name="ps3", bufs=1, space="PSUM") as PS3, \
                 tc.tile_pool(name="sb3", bufs=1) as SB3:
                from collections import deque
                pending = deque()   # deferred normalization pipeline stages

                def pop_pending(k=2):
                    n = 0
                    while pending and n < k:
                        s = pending.popleft()
                        if s is not None:
                            s()
                        n += 1

                # chunk list: (i, w); w=0 -> query cols [lo,512) (i<4 only),
                # w=1 -> [max(lo,512), 1024). L chunks first so the L-half
                # finishes early - its normalization frees the avs L banks
                # for the interleaved qk jobs with slack to spare.
                chunks = [(i, 0) for i in range(4)] + \
                         [(i, 1) for i in range(NT)]

                def chunk_cols(i, w):
                    lo = 128 * i
                    if w == 0:
                        return lo, 512
                    return max(lo, 512), T

                def emit_qk_job(tp, jidx, tags):
                    # one [128,512] slice of q^T (jidx 0/1) or k^T (2/3) for
                    # target head pair tp; the accumulator borrows an
                    # avs-tagged PSUM bank (free between L-normalization and
                    # the next pair's AV allocation)
                    m = tp if jidx < 2 else NC_ + tp
                    j2 = jidx % 2
                    acc = PS3.tile([128, 512], f32, tag=tags[jidx], bufs=1,
                                   name="qka")
                    for c in range(NC_):
                        nc.tensor.matmul(
                            acc[:], wQK[c][:, 128 * m:128 * (m + 1)],
                            xT[c][:, 512 * j2:512 * (j2 + 1)],
                            start=(c == 0), stop=(c == NC_ - 1))
                    # psum -> sbuf(bf16) with per-partition bias add
                    nc.vector.tensor_scalar_add(
                        qkT[m][:, 512 * j2:512 * (j2 + 1)], acc[:],
                        bqkt[:, m:m + 1])

                sps = {}    # (hp, ch) -> score PSUM tile
                pbs = {}    # (hp, ch) -> exp'd SBUF tile
                avst = {}   # (hp, hs, half) -> [128,512] accumulator

                def emit_score(hp, ch):
                    i, w = ch
                    lo = 128 * i
                    c0, c1 = chunk_cols(i, w)
                    qt = qkT[hp]
                    kt = qkT[NC_ + hp]
                    scp = PS3.tile([128, 1024], f32, tag="sc", bufs=2,
                                   name="scp")
                    for hs in range(2):
                        base = 64 * hs
                        nc.tensor.matmul(
                            scp[:, 512 * hs:512 * hs + (c1 - c0)],
                            kt[base:base + 64, lo:lo + 128],
                            qt[base:base + 64, c0:c1],
                            start=True, stop=True)
                    sps[(hp, ch)] = scp

                def emit_exp(hp, ch):
                    i, w = ch
                    lo = 128 * i
                    c0, c1 = chunk_cols(i, w)
                    n = c1 - c0
                    scp = sps.pop((hp, ch))
                    pb = SB3.tile([128, 1024], bf16, tag="pb", bufs=4,
                                  name="pb")
                    scv = scp.rearrange("p (s n) -> p s n", s=2)
                    pbv = pb.rearrange("p (s n) -> p s n", s=2)
                    nc.scalar.activation(pbv[:, :, 0:n], scv[:, :, 0:n],
                                         EXP, scale=0.125)
                    if c0 == lo:
                        # diagonal [128,128] block (both heads): DVE multiply
                        # by the precomputed causal mask
                        nc.vector.tensor_tensor(
                            pbv[:, :, 0:128], pbv[:, :, 0:128],
                            m2v[:], mybir.AluOpType.mult)
                    pbs[(hp, ch)] = pb

                def emit_av(hp, ch):
                    i, w = ch
                    c0, c1 = chunk_cols(i, w)
                    n = c1 - c0
                    pb = pbs.pop((hp, ch))
                    if i == 0 and w == 0:
                        for hs in range(2):
                            for half in range(2):
                                avst[(hp, hs, half)] = PS3.tile(
                                    [128, 512], f32, tag=f"av{hs}{half}",
                                    bufs=1, name=f"av{hs}{half}")
                    for hs in range(2):
                        h = 2 * hp + hs
                        vt = vA[i][:, 128 * h:128 * h + 128]
                        if w == 0:
                            nc.tensor.matmul(
                                avst[(hp, hs, 0)][:, c0:512], vt,
                                pb[:, 512 * hs:512 * hs + n],
                                start=(i == 0), stop=(i == 3),
                                skip_group_check=True)
                        else:
                            nc.tensor.matmul(
                                avst[(hp, hs, 1)][:, c0 - 512:512], vt,
                                pb[:, 512 * hs:512 * hs + n],
                                start=(i == 0), stop=(i == NT - 1),
                                skip_group_check=True)

                def make_norm(hp, half):
                    # avs rows 0:64 hold l replicated across partitions
                    # (ones-columns in vA), rows 64:128 hold y.
                    # reciprocal + normalize read PSUM directly - no
                    # staging copies, no DMA gathers/broadcasts.
                    loc = {}

                    def s_recip():
                        for hs in range(2):
                            rli = SB3.tile([64, 512], f32,
                                           tag=f"rli{hs}{half}", bufs=2,
                                           name=f"rli{hs}{half}")
                            nc.vector.reciprocal_approx_fast(
                                rli[:],
                                avst[(hp, hs, half)][0:64, 0:512])
                            loc[hs] = rli

                    def s_mult():
                        for hs in range(2):
                            base = 64 * hs
                            nc.vector.tensor_tensor(
                                yT[hp][base:base + 64,
                                       512 * half:512 * (half + 1)],
                                avst[(hp, hs, half)][64:128, 0:512],
                                loc[hs][:],
                                mybir.AluOpType.mult)

                    return [s_recip, s_mult]

                # prelude for head pair 0: the L-half chunks only need the
                # j2=0 (query/key cols < 512) jobs, so those go first and
                # the first score/exp pairs interleave with jobs 1 and 3
                JTAGS = ["av00", "av10", "av00", "av10"]
                nch = len(chunks)
                emit_qk_job(0, 0, ["av00"] * 4)
                emit_qk_job(0, 2, ["av10"] * 4)
                emit_score(0, chunks[0])
                emit_exp(0, chunks[0])
                emit_qk_job(0, 1, ["av01"] * 4)
                emit_score(0, chunks[1])
                emit_exp(0, chunks[1])
                emit_qk_job(0, 3, ["av11"] * 4)

                for hp in range(NHP):
                    if hp == 0:
                        # W_proj loads: sync-queue only (a scalar-queue DMA
                        # here would head-block the exp stream)
                        for c in range(NC_):
                            nc.sync.dma_start(
                                out=wpt[c][:],
                                in_=wp_d[128 * c:128 * (c + 1), :])
                    # steady state: scores TWO chunks ahead of the AV
                    # stream so the in-order PE queue always has a score MM
                    # to run while AV(n) waits on exp/affine(n). The first
                    # two score/exp pairs were emitted in the previous
                    # pair's tail (or the prelude).
                    for n_ in range(2, nch):
                        pop_pending()
                        emit_score(hp, chunks[n_])
                        emit_exp(hp, chunks[n_])
                        emit_av(hp, chunks[n_ - 2])
                        if n_ == 5:
                            # L-half normalization inline: the avs L banks
                            # are re-tagged as qk accumulators right after
                            for s in make_norm(hp, 0):
                                s()
                        elif n_ in (6, 8, 10) and hp + 1 < NHP:
                            emit_qk_job(hp + 1, {6: 0, 8: 1, 10: 2}[n_],
                                        JTAGS)
                    pop_pending()
                    emit_av(hp, chunks[nch - 2])
                    emit_av(hp, chunks[nch - 1])
                    if hp + 1 < NHP:
                        # job3 first: its eviction gates the next pair's
                        # avs allocation (same av10 tag), so give it the
                        # warmup's worth of slack
                        emit_qk_job(hp + 1, 3, JTAGS)
                        # next pair's warmup in this tail: keeps ACT fed
                        # across the pair boundary (its inputs - the j2=0
                        # qk jobs - completed mid-pair)
                        emit_score(hp + 1, chunks[0])
                        emit_exp(hp + 1, chunks[0])
                        emit_score(hp + 1, chunks[1])
                        emit_exp(hp + 1, chunks[1])
                    pending.extend(make_norm(hp, 1))

                # drain the tail (last pair's normalization)
                while pending:
                    s = pending.popleft()
                    if s is not None:
                        s()

            # ---- phase C: out = y^T.T @ W_proj + b_proj ----
            with tc.tile_pool(name="ps4", bufs=2, space="PSUM") as PS4, \
                 tc.tile_pool(name="sb4", bufs=3) as SB4:
                for t in range(NT):
                    acc = PS4.tile([128, C], f32, tag="pj", name="acc")
                    for c in range(NC_):
                        ycol = yT[c][:, 128 * t:128 * (t + 1)]
                        nc.tensor.matmul(acc[:, 0:512], ycol,
                                         wpt[c][:, 0:512],
                                         start=(c == 0), stop=(c == NC_ - 1))
                        nc.tensor.matmul(acc[:, 512:C], ycol,
                                         wpt[c][:, 512:C],
                                         start=(c == 0), stop=(c == NC_ - 1))
                    ot = SB4.tile([128, C], f32, tag="ot", bufs=3, name="ot")
                    nc.vector.tensor_copy(ot[:], acc[:])
                    nc.sync.dma_start(out=out_d[128 * t:128 * (t + 1), :],
                                      in_=ot[:])

    return nc


_WAIT_SKIP = {"InstNoOp", "InstEventSemOp", "InstSemaphoreOp",
              "InstPartitionBroadcast", "InstPartitionAllReduce"}


def _legalize_waits(nc):
    """walrus's codegen allows limited sync-wait commands per ISA struct
    (e.g. a Matmult's waits all land on the generated LDWEIGHTS struct which
    has one slot). Move excess waits onto same-engine NoOps inserted
    immediately before the instruction - program order on the engine queue
    preserves the synchronization semantics."""
    nfix = 0
    for fn in nc.m.functions:
        for bb in fn.blocks:
            out = []
            for ins in bb.instructions:
                si = ins.sync_info
                if (type(ins).__name__ not in _WAIT_SKIP and si is not None
                        and si.on_wait and len(si.on_wait) > 1):
                    waits = list(si.on_wait)
                    extra, keep = waits[:-1], waits[-1:]
                    for k, w in enumerate(extra):
                        nop = mybir.InstNoOp(name=f"{ins.name}-wf{k}", ins=[],
                                             outs=[])
                        nop.engine = ins.engine
                        nop.sync_info = mybir.SyncInfo(on_wait=[w],
                                                       on_update=[])
                        out.append(nop)
                    ins.sync_info = mybir.SyncInfo(
                        on_wait=keep, on_update=list(si.on_update or []))
                    nfix += 1
                out.append(ins)
            bb.instructions = out
    return nfix


_cached_module = None


def _get_module():
    global _cached_module
    if _cached_module is None:
        nc = build_module()
        # populate .instr bytes for InstCustomDveAnt (reciprocal_approx_fast)
        # - Bacc.compile() runs this pass but the raw-Bass path doesn't, and
        # walrus codegen fails with "ISA wrong length" on empty .instr
        mybir.codegen_inst_isa_subclasses(nc)
        _legalize_waits(nc)
        _cached_module = nc
    return _cached_module


def make_in_maps(x, W_attn, b_attn, W_proj, b_proj):
    import ml_dtypes
    bf = ml_dtypes.bfloat16
    x = np.asarray(x, dtype=np.float32)
    wa = np.asarray(W_attn, dtype=np.float32)
    wqk = np.ascontiguousarray(wa[:, 0:2 * C].astype(bf))
    wv = np.ascontiguousarray(wa[:, 2 * C:3 * C].astype(bf))
    wp = np.ascontiguousarray(np.asarray(W_proj, dtype=np.float32).astype(bf))
    ba = np.asarray(b_attn, dtype=np.float32)
    # q/k bias transposed on the host: bqkT[p, m] = b_attn[128*m + p],
    # loaded in one clean DMA and read as per-partition scalar columns
    bqkT = np.ascontiguousarray(ba[0:2 * C].reshape(2 * NC_, 128).T)
    return [
        dict(xT=np.ascontiguousarray(x[b].T.astype(bf)),
             Wqk=wqk, Wv=wv, Wp=wp, b_qkT=bqkT)
        for b in range(x.shape[0])
    ]


def run(x, W_attn, b_attn, W_proj, b_proj, trace=False, **spmd_kwargs):
    nc = _get_module()
    in_maps = make_in_maps(x, W_attn, b_attn, W_proj, b_proj)
    res = run_bass_kernel_spmd(nc, in_maps, list(range(NCORES)), trace=trace,
                               **spmd_kwargs)
    out = np.stack([res.results[b]["out"] for b in range(len(in_maps))],
                   axis=0)
    # the v bias passes through softmax exactly (attention weights sum to
    # 1), so it and b_proj fold into one constant output row added here
    ba = np.asarray(b_attn, dtype=np.float64)
    row = ba[2 * C:3 * C] @ np.asarray(W_proj, dtype=np.float64) \
        + np.asarray(b_proj, dtype=np.float64)
    out = out + row.astype(np.float32)[None, None, :]
    return out, res


def kernel(x, W_attn, b_attn, W_proj, b_proj):
    out, _ = run(x, W_attn, b_attn, W_proj, b_proj)
    return out
